# revision 26
# baseline (speedup 1.0000x reference)
"""CrossAttention Trainium2 Bass kernel (8 NeuronCores).

Problem (fp32): x [4, 2048, 1024], y [4, 2048, 768]
  q = x@Wq + bq; k = y@Wk + bk; v = y@Wv + bv           (16 heads x d_head 64)
  out = softmax(q k^T / 8) v  reshaped, then @ Wo + bo  -> [4, 2048, 1024]

Sharding: 8 cores = 4 batches x 2 head-halves. Core c handles batch c//2 and
heads (c%2)*8 .. +8 (d-slice of 512) for the full sequence, producing a
partial output^T [1024, 2048] = (Wo[d_slice, :])^T-matmuls over attn_half
(+ bo on half 0).  Host sums + transposes the two partials per batch.
No duplicated FLOPs, no collectives.

v2 (this file): all SBUF operands bf16 (PSUM accumulation stays fp32) so
weight loads ride the fast-weight-load path and DMA halves; phase D is
mac-outer/p-inner with the output projection (E) interleaved per mac so its
matmuls fill the PE slack under the ACT-bound exp stream; softmax
normalization uses reciprocal_approx_fast (denominators are sums of positive
exps, far from the undefined edge cases) + an fp32 ones-broadcast matmul.

Per-core dataflow:
  KT[d_half 512, sk 2048]  = (Wk_half)^T @ y^T   (+bk)     bf16
  v' [sk 2048, 8*65]       = y @ Wv_aug (+bv_aug); 65th col per head = 1.0
                             (ones column makes the AV matmul emit softmax
                              denominators for free)
  qT[d_half 512, sq 2048]  = (Wq_half)^T @ x^T   (+bq)     bf16
  per (sq-macro 512, head pair), software-pipelined over 8 groups of 2 sk:
    scoresT half-tiles per head via row-packed K=64 matmuls (2 heads concur.)
    expT = ACT exp(scoresT / 8) -> bf16, [128,1024] calls, double-buffered
      score PSUM so ACT streams continuously
    out'[65, 512] accumulates v'^T-matmuls over 16 sk tiles (row 64 = denom);
      AV of group g is emitted after scores/exp of group g+1 so the in-order
      PE stream never blocks the next exp
    attn = out'[0:64] * bcast(1/denom)  (fp32 K=1 ones matmul + DVE mul)
  after the 4 head pairs of each sq-macro:
    outT[:, macro] += (Wo_half slice)^T-matmuls over attn (+bo per-partition)
"""

import numpy as np

import concourse.bass as bass
import concourse.mybir as mybir
import concourse.tile as tile
from concourse.bass_utils import run_bass_kernel_spmd

F32 = mybir.dt.float32
BF16 = mybir.dt.bfloat16
AF = mybir.ActivationFunctionType
ALU = mybir.AluOpType

B, S, DE, DC = 4, 2048, 1024, 768
H, DH = 16, 64
HH = H // 2          # heads per core
DHALF = DE // 2      # 512, d-slice per core
NMT = DHALF // 128   # 4 head pairs
NKT_X = DE // 128    # 8 k-tiles for q projection
NKT_Y = DC // 128    # 6 k-tiles for k/v projections
NSK = S // 128       # 16 sk tiles
NMAC = S // 512      # 4 sq macros
NDE = DE // 128      # 8 output-column tiles
VW = DH + 1          # 65 cols per head in v' (ones column at 64)
VTOT = HH * VW       # 520
SCALE = 1.0 / np.sqrt(DH)

_prog_cache = {}


def _split_sync_waits(nc):
    """This container's walrus accepts only 1 sync wait per instruction.
    Tile attaches one wait per producer proc. For every instruction with k>1
    waits, insert k-1 single-wait nops on the same engine right before it
    (equivalent semantics: the engine's stream waits serially)."""
    eng_map = {
        mybir.EngineType.PE: nc.tensor,
        mybir.EngineType.Activation: nc.scalar,
        mybir.EngineType.DVE: nc.vector,
        mybir.EngineType.Pool: nc.gpsimd,
        mybir.EngineType.SP: nc.sync,
    }
    for bb in nc.main_func.blocks:
        insts = bb.instructions
        fixes = []
        for idx, ins in enumerate(insts):
            si = ins.sync_info
            if si and si.on_wait and len(si.on_wait) > 1:
                fixes.append((idx, ins))
        for idx, ins in reversed(fixes):
            eng = eng_map.get(ins.engine)
            if eng is None:
                continue
            waits = list(ins.sync_info.on_wait)
            ins.sync_info.on_wait = [waits[-1]]
            nops = []
            for w in waits[:-1]:
                n = eng.nop(nofuse=True).ins
                for b2 in nc.main_func.blocks:
                    if b2.instructions and b2.instructions[-1] is n:
                        b2.instructions.pop()
                        break
                n.sync_info = mybir.SyncInfo(on_wait=[w], on_update=[])
                nops.append(n)
            for j, n in enumerate(nops):
                insts.insert(idx + j, n)
    return nc


def _act_recip(nc, out_ap, in_ap):
    """ACT-engine reciprocal via raw InstActivation (the bass wrapper's
    accuracy guard is for edge ranges; softmax denominators are sums of
    positive exps, probed max rel err 1.2e-5 on [1, 5e3])."""
    act = nc.scalar
    ins = [act.lower_ap(in_ap)]
    for arg in (0.0, 1.0, 0.0):  # bias, scale, alpha
        ins.append(mybir.ImmediateValue(dtype=mybir.dt.float32, value=arg))
    return act.add_instruction(
        mybir.InstActivation(
            name=nc.get_next_instruction_name(),
            func=mybir.ActivationFunctionType.Reciprocal,
            ins=ins,
            outs=[act.lower_ap(out_ap)],
        )
    )


def build_program(n_reps: int = 1, upto: str = 'E'):
    nc = bass.Bass()

    xT = nc.dram_tensor("xT", [DE, S], BF16, kind="ExternalInput")
    yT = nc.dram_tensor("yT", [DC, S], BF16, kind="ExternalInput")
    wq = nc.dram_tensor("wq", [DE, DHALF], BF16, kind="ExternalInput")
    wk = nc.dram_tensor("wk", [DC, DHALF], BF16, kind="ExternalInput")
    wv = nc.dram_tensor("wv", [DC, VTOT], BF16, kind="ExternalInput")
    wo = nc.dram_tensor("wo", [DHALF, DE], BF16, kind="ExternalInput")
    bqd = nc.dram_tensor("bq", [128, NMT], F32, kind="ExternalInput")
    bkd = nc.dram_tensor("bk", [128, NMT], F32, kind="ExternalInput")
    bvd = nc.dram_tensor("bv", [128, VTOT], BF16, kind="ExternalInput")
    bod = nc.dram_tensor("bo", [128, NDE], F32, kind="ExternalInput")
    outd = nc.dram_tensor("out", [DE, S], F32, kind="ExternalOutput")
    # per-(mac,p,j) scratch rows for the 1/denom DMA partition-broadcast
    scrd = nc.dram_tensor("scr", [NMAC * NMT * 2, 512], BF16, kind="Internal")

    from contextlib import ExitStack

    with tile.TileContext(nc) as tc:
      for _rep in range(n_reps):  # >1 only for timing (amortizes dispatch)
        with ExitStack() as ctx:
            pconst = ctx.enter_context(tc.tile_pool(name="const", bufs=1))
            bv_sb = pconst.tile([128, VTOT], BF16, name="bv")
            bo_sb = pconst.tile([128, NDE], F32, name="bo")
            bq_sb = pconst.tile([128, NMT], F32, name="bq")
            bk_sb = pconst.tile([128, NMT], F32, name="bk")
            nc.sync.dma_start(bv_sb[:], bvd[:])
            nc.sync.dma_start(bo_sb[:], bod[:])
            nc.sync.dma_start(bq_sb[:], bqd[:])
            nc.sync.dma_start(bk_sb[:], bkd[:])

            # persistent activations
            pqT = ctx.enter_context(tc.tile_pool(name="qT", bufs=NMT))
            qT = [pqT.tile([128, S], BF16, name="qT") for _ in range(NMT)]
            pKT = ctx.enter_context(tc.tile_pool(name="KT", bufs=NMT))
            KT = [pKT.tile([128, S], BF16, name="KT") for _ in range(NMT)]
            pv = ctx.enter_context(tc.tile_pool(name="v", bufs=NSK))
            vsb = [pv.tile([128, VTOT], BF16, name="v") for _ in range(NSK)]
            pattn = ctx.enter_context(tc.tile_pool(name="attn", bufs=NMT))
            attn = [pattn.tile([128, S], BF16, name="attn") for _ in range(NMT)]
            pwo = ctx.enter_context(tc.tile_pool(name="wop", bufs=NMT))

            # ---- Phases B+C: KT and v' from yT; then A: qT from xT ----
            with (
                tc.tile_pool(name="yTp", bufs=NKT_Y) as pyT,
                tc.tile_pool(name="xTp", bufs=NKT_X) as pxT,
                tc.tile_pool(name="wqp", bufs=NKT_X) as pwq,
                tc.tile_pool(name="wkp", bufs=NKT_Y) as pwk,
                tc.tile_pool(name="wvp", bufs=NKT_Y) as pwv,
            ):
                # gating loads first (wk, then y), split across both queues
                wkt = []
                for kt in range(NKT_Y):
                    t = pwk.tile([128, DHALF], BF16, name="wkt")
                    q = nc.gpsimd if kt % 2 == 0 else nc.sync
                    q.dma_start(t[:], wk[kt * 128 : (kt + 1) * 128, :])
                    wkt.append(t)
                yt = []
                for kt in range(NKT_Y):
                    t = pyT.tile([128, S], BF16, name="yt")
                    q = nc.sync if kt % 2 == 0 else nc.gpsimd
                    q.dma_start(t[:], yT[kt * 128 : (kt + 1) * 128, :])
                    yt.append(t)
                wvt = []
                for kt in range(NKT_Y):
                    t = pwv.tile([128, VTOT], BF16, name="wvt")
                    q = nc.gpsimd if kt % 2 == 0 else nc.sync
                    q.dma_start(t[:], wv[kt * 128 : (kt + 1) * 128, :])
                    wvt.append(t)
                # x / wq / wo loads follow, overlapped with B/C compute
                xt = []
                wqt = []
                for kt in range(NKT_X):
                    t = pxT.tile([128, S], BF16, name="xt")
                    nc.gpsimd.dma_start(t[:], xT[kt * 128 : (kt + 1) * 128, :])
                    xt.append(t)
                    t = pwq.tile([128, DHALF], BF16, name="wqt")
                    nc.gpsimd.dma_start(t[:], wq[kt * 128 : (kt + 1) * 128, :])
                    wqt.append(t)
                wot = []
                for kt in range(NMT):
                    t = pwo.tile([128, DE], BF16, name="wot")
                    nc.sync.dma_start(t[:], wo[kt * 128 : (kt + 1) * 128, :])
                    wot.append(t)

                with tc.tile_pool(name="psB", bufs=4, space="PSUM") as psB:
                    for mt in range(NMT):
                        for nn in range(NMAC):
                            ps = psB.tile([128, 512], F32, name="psB")
                            for kt in range(NKT_Y):
                                nc.tensor.matmul(
                                    ps[:],
                                    wkt[kt][:, mt * 128 : (mt + 1) * 128],
                                    yt[kt][:, nn * 512 : (nn + 1) * 512],
                                    start=(kt == 0),
                                    stop=(kt == NKT_Y - 1),
                                )
                            with nc.allow_low_precision(reason="bf16 store"):
                                nc.vector.tensor_scalar(
                                    KT[mt][:, nn * 512 : (nn + 1) * 512],
                                    ps[:],
                                    bk_sb[:, mt : mt + 1],
                                    None,
                                    ALU.add,
                                )
                with tc.tile_pool(name="psC", bufs=4, space="PSUM") as psC:
                    for sk in range(NSK):
                        for nn2 in range(2):
                            lo, hi = nn2 * 260, nn2 * 260 + 260
                            ps = psC.tile([128, 260], F32, name="psC")
                            for kt in range(NKT_Y):
                                nc.tensor.matmul(
                                    ps[:],
                                    yt[kt][:, sk * 128 : (sk + 1) * 128],
                                    wvt[kt][:, lo:hi],
                                    start=(kt == 0),
                                    stop=(kt == NKT_Y - 1),
                                )
                            # bias add via pre-broadcast bv tile (no PE work)
                            with nc.allow_low_precision(reason="bf16 store"):
                                nc.vector.tensor_add(
                                    vsb[sk][:, lo:hi], ps[:], bv_sb[:, lo:hi]
                                )

                # ---- Phase A: qT = Wq^T @ xT (+bq) ----
                with tc.tile_pool(name="psA", bufs=4, space="PSUM") as psA:
                    for mt in range(NMT):
                        for nn in range(NMAC):
                            ps = psA.tile([128, 512], F32, name="psA")
                            for kt in range(NKT_X):
                                nc.tensor.matmul(
                                    ps[:],
                                    wqt[kt][:, mt * 128 : (mt + 1) * 128],
                                    xt[kt][:, nn * 512 : (nn + 1) * 512],
                                    start=(kt == 0),
                                    stop=(kt == NKT_X - 1),
                                )
                            with nc.allow_low_precision(reason="bf16 store"):
                                nc.vector.tensor_scalar(
                                    qT[mt][:, nn * 512 : (nn + 1) * 512],
                                    ps[:],
                                    bq_sb[:, mt : mt + 1],
                                    None,
                                    ALU.add,
                                )

            # ---- Phase D + E interleaved: attention then out-projection ----
            if upto == 'C':
                nc.gpsimd.dma_start(outd[0:128, :], qT[0][:, 0:2048])
                continue
            with (
                tc.tile_pool(name="expp", bufs=16) as pexp,
                tc.tile_pool(name="normp", bufs=4) as pnorm,
                tc.tile_pool(name="sb65p", bufs=4) as p65,
                tc.tile_pool(name="outsb", bufs=3) as pout,
                tc.tile_pool(name="scps", bufs=3, space="PSUM") as pssc,
                tc.tile_pool(name="avps", bufs=2, space="PSUM") as psav,
            ):
                AVLAG = 4  # AV trails scores by this many groups so the PE
                # queue has runway while the previous iteration's softmax
                # normalization chain (DVE recip + DMA bcast + mul) drains

                def emit_e_tile(sq_, n):
                    # one outT tile of phase E; reuses a scores-pool slot
                    # (same name+shape); only bank 0 (cols 0:512) is written
                    def f():
                        ps = pssc.tile([128, 1024], F32, name="scps")
                        for kt in range(NMT):
                            nc.tensor.matmul(
                                ps[:, 0:512],
                                wot[kt][:, n * 128 : (n + 1) * 128],
                                attn[kt][:, sq_ : sq_ + 512],
                                start=(kt == 0),
                                stop=(kt == NMT - 1),
                            )
                        osb = pout.tile([128, 512], F32, name="osb")
                        nc.vector.tensor_scalar(
                            osb[:], ps[:, 0:512], bo_sb[:, n : n + 1], None,
                            ALU.add,
                        )
                        nc.sync.dma_start(
                            outd[n * 128 : (n + 1) * 128, sq_ : sq_ + 512],
                            osb[:],
                        )
                    return f

                pending_e = []
                for mac in range(NMAC):
                    sq = mac * 512
                    for p in range(NMT):
                        outp = [
                            psav.tile([VW, 512], F32, name="avps") for _ in range(2)
                        ]

                        def emit_av(item):
                            pa, pb, pg = item
                            for j, ex in ((0, pa), (1, pb)):
                                lh = 2 * p + j
                                for d2 in range(2):
                                    t = 2 * pg + d2
                                    nc.tensor.matmul(
                                        outp[j][:],
                                        vsb[t][:, lh * VW : (lh + 1) * VW],
                                        ex[:, d2 * 512 : (d2 + 1) * 512],
                                        start=(t == 0),
                                        stop=(t == NSK - 1),
                                    )

                        exq = []
                        for g in range(NSK // 2):
                            # j-interleaved emission: consecutive score MMs
                            # alternate PE row groups (0-63 / 64-127) so the
                            # K=64 matmuls can overlap in the array
                            scs = [
                                pssc.tile([128, 1024], F32, name="scps")
                                for _ in range(2)
                            ]
                            for d2 in range(2):
                                t = 2 * g + d2
                                for j in range(2):
                                    nc.tensor.matmul(
                                        scs[j][:, d2 * 512 : (d2 + 1) * 512],
                                        KT[p][
                                            j * 64 : j * 64 + 64,
                                            t * 128 : (t + 1) * 128,
                                        ],
                                        qT[p][j * 64 : j * 64 + 64, sq : sq + 512],
                                        start=True,
                                        stop=True,
                                    )
                            exps = []
                            for j in range(2):
                                ex = pexp.tile([128, 1024], BF16, name="expt")
                                nc.scalar.activation(
                                    ex[:], scs[j][:], AF.Exp, scale=SCALE
                                )
                                exps.append(ex)
                            exq.append((exps[0], exps[1], g))
                            if len(exq) > AVLAG:
                                emit_av(exq.pop(0))
                            # previous sq-macro's out-projection tiles fill
                            # the PE slack under the ACT-bound exp stream
                            # (g>=4 so the attn normalization chain has
                            # drained by the time the first E tile runs)
                            if g >= 4 and pending_e:
                                pending_e.pop(0)()
                        while exq:
                            emit_av(exq.pop(0))
                        # free BOTH PSUM banks right away (copies first), so
                        # the next iteration's AV never waits on the rest of
                        # the normalization chain
                        sb65s = []
                        for j in range(2):
                            sb65 = p65.tile([VW, 512], BF16, name="sb65")
                            with nc.allow_low_precision(reason="bf16 store"):
                                nc.vector.tensor_copy(sb65[:], outp[j][:])
                            sb65s.append(sb65)
                        bcss = []
                        for j in range(2):
                            slot = (mac * NMT + p) * 2 + j
                            rd = pnorm.tile([1, 512], BF16, name="rd")
                            with nc.allow_low_precision(reason="bf16 store"):
                                nc.vector.reciprocal(rd[:], sb65s[j][64:65, :])
                            # broadcast 1/denom across partitions via a DRAM
                            # round-trip (0-stride DRAM read); j0/j1 ride
                            # different DMA queues
                            q = nc.gpsimd if j == 0 else nc.sync
                            scr = scrd[slot : slot + 1, :]
                            q.dma_start(scr, rd[:])
                            bcs = pnorm.tile([64, 512], BF16, name="bcs")
                            q.dma_start(bcs[:], scr.partition_broadcast(64))
                            bcss.append(bcs)
                        for j in range(2):
                            if j == 0:
                                with nc.allow_low_precision(reason="bf16 store"):
                                    nc.vector.tensor_mul(
                                        attn[p][0:64, sq : sq + 512],
                                        sb65s[j][0:64, :],
                                        bcss[j][:],
                                    )
                            else:
                                tmp = pnorm.tile([64, 512], BF16, name="tmpn")
                                with nc.allow_low_precision(reason="bf16 store"):
                                    nc.vector.tensor_mul(
                                        tmp[:], sb65s[j][0:64, :], bcss[j][:]
                                    )
                                # DVE lanes cannot shift partitions; DMA moves
                                # the odd head's rows to partitions 64..127
                                nc.sync.dma_start(
                                    attn[p][64:128, sq : sq + 512], tmp[:]
                                )

                    # ---- queue phase E for this sq-macro ----
                    if upto == 'D':
                        continue
                    pending_e.extend(emit_e_tile(sq, n) for n in range(NDE))
                while pending_e:
                    pending_e.pop(0)()
                if upto == 'D':
                    nc.gpsimd.dma_start(outd[0:128, :], attn[0][:, 0:2048])

    return _split_sync_waits(nc)


def _to_bf16(a):
    import ml_dtypes

    return np.ascontiguousarray(a.astype(ml_dtypes.bfloat16))


def _host_prep(x, y, Wq, bq, Wk, bk, Wv, bv, Wo, bo):
    x = np.asarray(x, dtype=np.float32)
    y = np.asarray(y, dtype=np.float32)
    Wq = np.asarray(Wq, dtype=np.float32)
    Wk = np.asarray(Wk, dtype=np.float32)
    Wv = np.asarray(Wv, dtype=np.float32)
    Wo = np.asarray(Wo, dtype=np.float32)
    bq = np.asarray(bq, dtype=np.float32)
    bk = np.asarray(bk, dtype=np.float32)
    bv = np.asarray(bv, dtype=np.float32)
    bo = np.asarray(bo, dtype=np.float32)
    in_maps = []
    for c in range(8):
        b, hh = c // 2, c % 2
        dlo = hh * DHALF
        wv_aug = np.zeros((DC, VTOT), dtype=np.float32)
        bv_aug = np.zeros((1, VTOT), dtype=np.float32)
        for lh in range(HH):
            gh = hh * HH + lh
            wv_aug[:, lh * VW : lh * VW + DH] = Wv[:, gh * DH : (gh + 1) * DH]
            bv_aug[0, lh * VW : lh * VW + DH] = bv[gh * DH : (gh + 1) * DH]
            bv_aug[0, lh * VW + DH] = 1.0
        in_maps.append(
            {
                "xT": _to_bf16(x[b].T),
                "yT": _to_bf16(y[b].T),
                "wq": _to_bf16(Wq[:, dlo : dlo + DHALF]),
                "wk": _to_bf16(Wk[:, dlo : dlo + DHALF]),
                "wv": _to_bf16(wv_aug),
                "wo": _to_bf16(Wo[dlo : dlo + DHALF, :]),
                "bq": np.ascontiguousarray(
                    bq[dlo : dlo + DHALF].reshape(NMT, 128).T
                ),
                "bk": np.ascontiguousarray(
                    bk[dlo : dlo + DHALF].reshape(NMT, 128).T
                ),
                "bv": _to_bf16(np.broadcast_to(bv_aug, (128, VTOT))),
                "bo": np.ascontiguousarray(
                    (bo if hh == 0 else np.zeros_like(bo)).reshape(NDE, 128).T
                ),
            }
        )
    return in_maps


def _gather(results):
    parts = [results[c]["out"] for c in range(8)]
    return np.stack(
        [
            np.ascontiguousarray(
                (parts[2 * b].astype(np.float32) + parts[2 * b + 1]).T
            )
            for b in range(B)
        ]
    )


def kernel(x, y, Wq, bq, Wk, bk, Wv, bv, Wo, bo, _results_out=None, _trace=False):
    if "nc" not in _prog_cache:
        _prog_cache["nc"] = build_program()
    nc = _prog_cache["nc"]
    in_maps = _host_prep(x, y, Wq, bq, Wk, bk, Wv, bv, Wo, bo)
    res = run_bass_kernel_spmd(nc, in_maps, core_ids=list(range(8)), trace=_trace)
    if _results_out is not None:
        _results_out.append(res)
    return _gather(res.results)
